# revision 7
# baseline (speedup 1.0000x reference)
"""LGEB (Lorentz-group-equivariant block) message-passing kernel for 8 Trainium2
NeuronCores.

Strategy (edges sharded by target node, nodes sharded 1/8 per core):
  - Host: sort edges by edge_i, shard by target-node range, pack into
    segment-aligned 512-edge blocks whose node span is <= 126.
  - P0 (device): per-node projection tables
      Atab[n] = [h[n] @ We1_hi + be1 (bf16, 96) | pad]          (own shard, 128 cols)
      Aqx[n]  = [minkowski q, eta*x (f32, 5) | pad]             (own shard)
      Btab[n] = [h[n] @ We1_hj (bf16 96) | psi pad 2 | q,x as f32-in-bf16 10 | pad 4]
      Btab built per-shard then AllGather'ed to all cores (50000 x 112 bf16).
  - P1 (device): per 512-edge block: transposed dma_gather of A rows, indirect
    gather of B rows, Minkowski invariants + psi on DVE/ACT, edge MLP as
    feature-major matmuls (features on partitions, edges on the free axis),
    segment-sum via one-hot matmul into PSUM, indirect scatter of per-node
    partial sums (each output row written exactly once).
  - P2 (device): node update MLP on the 1/8 shard; outputs h_out/x_out shards.
Host reassembles the 8 shards.
"""

import numpy as np
import ml_dtypes

N_NODES = 50000
N_EDGES = 800000
C_FEAT = 96
HID = 96
C_CONST = 0.005
NCORES = 8
EB = 512           # edges per block
NP = 4             # 128-edge pieces per block
SENT = 127         # pad-edge local id (psum row 127 is never scattered)
OOB = 1 << 20      # scatter index for "skip this row"

import concourse.bass as bass
import concourse.mybir as mybir
import concourse.tile as tile
from concourse import bacc, bass_utils
from concourse.tile import add_dep_helper
from concourse.masks import make_identity

f32 = mybir.dt.float32
bf16 = mybir.dt.bfloat16
i32 = mybir.dt.int32
i16 = mybir.dt.int16
AF = mybir.ActivationFunctionType
ALU = mybir.AluOpType


# ---------------------------------------------------------------- host side

def _pack_blocks(ei_s, ej_s, lo_e, hi_e, base, nsh):
    """Greedy segment-aligned blocks of <=EB edges with node span <=126.
    Returns list of (s, e, lo_node)."""
    blocks = []
    s = lo_e
    while s < hi_e:
        e = min(s + EB, hi_e)
        if e < hi_e:
            # backtrack to the start of the segment containing edge e
            e2 = int(np.searchsorted(ei_s[s:e + 1], ei_s[e], side="left")) + s
            if e2 > s:
                e = e2
            # else: single segment longer than EB; keep e (split segment) --
            # cannot happen for this distribution, guarded by assert below
            assert e2 > s, "segment with >512 edges not supported"
        while ei_s[e - 1] - ei_s[s] > 126:
            limit = ei_s[s] + 126
            e = int(np.searchsorted(ei_s[s:e], limit, side="right")) + s
        blocks.append((s, e, int(ei_s[s])))
        s = e
    return blocks


def build_host_data(x, h, edge_i, edge_j, n_nodes=N_NODES, ncores=NCORES):
    nsh = n_nodes // ncores
    order = np.argsort(edge_i, kind="stable")
    ei_s = edge_i[order].astype(np.int64)
    ej_s = edge_j[order].astype(np.int64)
    cuts = np.searchsorted(ei_s, np.arange(0, n_nodes + 1, nsh))

    per_core_blocks = []
    for c in range(ncores):
        blks = _pack_blocks(ei_s, ej_s, int(cuts[c]), int(cuts[c + 1]),
                            c * nsh, nsh)
        per_core_blocks.append(blks)
    nb = max(len(b) for b in per_core_blocks)

    cores = []
    for c in range(ncores):
        base = c * nsh
        blks = per_core_blocks[c]
        agidx = np.zeros((nb, 128, EB // 16), np.int16)
        meta = np.full((nb, 128, 9), 0, np.int32)
        meta[:, :, 8] = OOB
        localf = np.full((nb, 128, NP), float(SENT), np.float32)

        # rows needing explicit zeros: shard nodes not covered by any block
        covered = np.zeros(nsh, bool)
        for (s, e, lo) in blks:
            covered[ei_s[s] - base: ei_s[e - 1] - base + 1] = True
        gap_rows = np.nonzero(~covered)[0].tolist()

        for b, (s, e, lo) in enumerate(blks):
            n_real = e - s
            ei_b = ei_s[s:e] - base          # shard-relative target ids
            ej_b = ej_s[s:e]
            loc_b = ei_s[s:e] - lo           # block-local ids in [0, 127)
            span = int(loc_b[-1])
            assert span <= 126

            # dma_gather idx packing: idx[i] at [i%16 (+16k replicas), i//16]
            full_idx = np.zeros(EB, np.int64)
            full_idx[:n_real] = ei_b
            w = full_idx.reshape(EB // 16, 16).T.astype(np.int16)  # [16, EB/16]
            agidx[b] = np.tile(w, (8, 1))

            for p in range(NP):
                sl = slice(p * 128, min((p + 1) * 128, n_real))
                ln = sl.stop - sl.start
                if ln <= 0:
                    continue
                meta[b, :ln, p] = ej_b[sl]
                meta[b, :ln, 4 + p] = ei_b[sl]
                localf[b, :ln, p] = loc_b[sl]

            scat = np.full(128, OOB, np.int64)
            scat[:span + 1] = (lo - base) + np.arange(span + 1)
            r = span + 1
            while r < 127 and gap_rows:
                scat[r] = gap_rows.pop()
                r += 1
            meta[b, :, 8] = scat
        assert not gap_rows, "ran out of spare scatter rows for zero-fill"
        cores.append(dict(agidx=agidx, meta=meta, localf=localf))
    return nb, nsh, cores


def _pack_weights(W):
    """Host-side packing of the tiny MLP weights into kernel layouts."""
    d = {}
    d["We1_hi"] = W["We1"][:C_FEAT].astype(np.float32)            # [96,96]
    d["We1_hj"] = W["We1"][C_FEAT:2 * C_FEAT].astype(np.float32)  # [96,96]
    d["wmisc_ext"] = W["We1"][2 * C_FEAT:].astype(ml_dtypes.bfloat16)  # [2,96]
    d["We2"] = W["We2"].astype(np.float32)
    d["Wx1Wm"] = np.concatenate([W["Wx1"], W["Wm"]], axis=1).astype(np.float32)  # [96,97]
    d["Wx2"] = W["Wx2"].astype(np.float32)                         # [96,1]
    d["Wh1h"] = W["Wh1"][:C_FEAT].astype(np.float32)
    d["Wh1wm"] = W["Wh1"][C_FEAT:].astype(np.float32)
    d["Wh2"] = W["Wh2"].astype(np.float32)
    d["be1"] = W["be1"].reshape(96, 1).astype(np.float32)
    d["be2"] = W["be2"].reshape(96, 1).astype(np.float32)
    d["bx1m"] = np.concatenate([W["bx1"], W["bm"]]).reshape(97, 1).astype(np.float32)
    d["bh1"] = W["bh1"].reshape(96, 1).astype(np.float32)
    d["bh2"] = W["bh2"].reshape(96, 1).astype(np.float32)
    sel = np.zeros((128, 96), np.float32)
    sel[:96, :96] = np.eye(96)
    d["selA"] = sel.astype(ml_dtypes.bfloat16)                     # [128,96] bf16
    d["iota_rows"] = np.broadcast_to(
        np.arange(128, dtype=np.float32), (128, 128)).copy()
    d["eta"] = np.broadcast_to(
        np.array([1.0, -1.0, -1.0, -1.0], np.float32), (128, 4)).copy()
    return d


WEIGHT_SPECS = [
    ("We1_hi", [96, 96], f32), ("We1_hj", [96, 96], f32),
    ("wmisc_ext", [2, 96], bf16), ("We2", [96, 96], f32),
    ("Wx1Wm", [96, 97], f32), ("Wx2", [96, 1], f32),
    ("Wh1h", [96, 96], f32), ("Wh1wm", [96, 96], f32), ("Wh2", [96, 96], f32),
    ("be1", [96, 1], f32), ("be2", [96, 1], f32), ("bx1m", [97, 1], f32),
    ("bh1", [96, 1], f32), ("bh2", [96, 1], f32),
    ("selA", [128, 96], bf16), ("iota_rows", [128, 128], f32),
    ("eta", [128, 4], f32),
]


# ---------------------------------------------------------------- device side

def build_nc(nb, nsh, n_nodes=N_NODES, ncores=NCORES):
    nc = bacc.Bacc("TRN2", target_bir_lowering=False, debug=False,
                   num_devices=ncores)

    h_sh = nc.dram_tensor("h_sh", [nsh, C_FEAT], f32, kind="ExternalInput")
    x_sh = nc.dram_tensor("x_sh", [nsh, 4], f32, kind="ExternalInput")
    agidx_in = nc.dram_tensor("agidx", [nb, 128, EB // 16], i16, kind="ExternalInput")
    meta_in = nc.dram_tensor("meta", [nb, 128, 9], i32, kind="ExternalInput")
    localf_in = nc.dram_tensor("localf", [nb, 128, NP], f32, kind="ExternalInput")
    Wt = {name: nc.dram_tensor(name, shp, dt, kind="ExternalInput")
          for name, shp, dt in WEIGHT_SPECS}

    h_out = nc.dram_tensor("h_out", [nsh, C_FEAT], f32, kind="ExternalOutput")
    x_out = nc.dram_tensor("x_out", [nsh, 4], f32, kind="ExternalOutput")

    Atab = nc.dram_tensor("Atab_int", [nsh, 128], bf16)
    Aqx = nc.dram_tensor("Aqx_int", [nsh, 8], f32)
    Bsh = nc.dram_tensor("Bsh_int", [nsh, 112], bf16)
    Btab = nc.dram_tensor("Btab_int", [n_nodes, 112], bf16, addr_space="Shared")
    hT_dram = nc.dram_tensor("hT_int", [C_FEAT, nsh], f32)
    wmsc = nc.dram_tensor("wmsc_int", [nsh, 101], f32)

    ntiles = (nsh + 127) // 128

    with tile.TileContext(nc) as tc:
        with (
            tc.tile_pool(name="const", bufs=1) as cpool,
            tc.tile_pool(name="sb", bufs=3) as sb,
            tc.tile_pool(name="sb1", bufs=2) as sb1,
            tc.tile_pool(name="ps", bufs=1, space="PSUM") as ps,
            tc.tile_pool(name="ps2", bufs=2, space="PSUM") as ps2,
        ):
            # ---- constants
            identF = cpool.tile([128, 128], f32, tag="identF")
            make_identity(nc, identF[:])
            identB = cpool.tile([128, 128], bf16, tag="identB")
            make_identity(nc, identB[:])
            W = {}
            for name, shp, dt in WEIGHT_SPECS:
                W[name] = cpool.tile(shp, dt, tag=name, name="w_" + name)
                nc.sync.dma_start(out=W[name][:], in_=Wt[name][:, :])

            # ================= P0: tables =================
            for t in range(ntiles):
                b0 = t * 128
                R = min(128, nsh - b0)
                h_t = sb.tile([128, C_FEAT], f32, tag="p0_h")
                nc.sync.dma_start(out=h_t[0:R, :], in_=h_sh[b0:b0 + R, :])
                x_t = sb.tile([128, 4], f32, tag="p0_x")
                nc.sync.dma_start(out=x_t[0:R, :], in_=x_sh[b0:b0 + R, :])

                psA = ps.tile([96, 128], f32, tag="ps1")
                nc.tensor.matmul(out=psA[:, 0:R], lhsT=h_t[0:R, :],
                                 rhs=identF[0:R, 0:R], is_transpose=True,
                                 start=True, stop=True)
                hT_sb = sb.tile([96, 128], f32, tag="p0_hT")
                nc.scalar.activation(out=hT_sb[:, 0:R], in_=psA[:, 0:R], func=AF.Copy)
                nc.sync.dma_start(out=hT_dram[:, b0:b0 + R], in_=hT_sb[:, 0:R])

                for (w_proj, w_bias, asm_tag, asm_cols, dram) in (
                    ("We1_hi", "be1", "p0_asmA", 128, Atab),
                    ("We1_hj", None, "p0_asmB", 112, Bsh),
                ):
                    psP = ps.tile([96, 128], f32, tag="ps2t")
                    nc.tensor.matmul(out=psP[:, 0:R], lhsT=W[w_proj][:],
                                     rhs=hT_sb[:, 0:R], start=True, stop=True)
                    pT = sb.tile([96, 128], f32, tag=asm_tag + "T")
                    if w_bias is not None:
                        nc.scalar.activation(out=pT[:, 0:R], in_=psP[:, 0:R],
                                             func=AF.Copy, bias=0.0,
                                             scale=1.0)
                        # bias added feature-major (per-partition)
                        nc.vector.tensor_tensor(
                            out=pT[:, 0:R], in0=pT[:, 0:R],
                            in1=W[w_bias][:].to_broadcast([96, R]), op=ALU.add)
                    else:
                        nc.scalar.activation(out=pT[:, 0:R], in_=psP[:, 0:R],
                                             func=AF.Copy)
                    psNM = ps.tile([128, 96], f32, tag="ps3t")
                    nc.tensor.matmul(out=psNM[0:R, :], lhsT=pT[:, 0:R],
                                     rhs=identF[0:96, 0:96], is_transpose=True,
                                     start=True, stop=True)
                    asm = sb.tile([128, asm_cols], bf16, tag=asm_tag)
                    nc.vector.memset(asm[:], 0.0)
                    nc.scalar.activation(out=asm[0:R, 0:96], in_=psNM[0:R, :],
                                         func=AF.Copy)
                    if dram is Atab:
                        nc.sync.dma_start(out=Atab[b0:b0 + R, :], in_=asm[0:R, :])
                    else:
                        # pack q and raw x as f32 into bf16 cols 98:108
                        xe = sb.tile([128, 4], f32, tag="p0_xe")
                        nc.vector.tensor_tensor(out=xe[0:R, :], in0=x_t[0:R, :],
                                                in1=W["eta"][0:R, :], op=ALU.mult)
                        xq = sb.tile([128, 4], f32, tag="p0_xq")
                        nc.vector.tensor_tensor(out=xq[0:R, :], in0=xe[0:R, :],
                                                in1=x_t[0:R, :], op=ALU.mult)
                        q = sb.tile([128, 1], f32, tag="p0_q")
                        nc.vector.reduce_sum(q[0:R, :], xq[0:R, :],
                                             axis=mybir.AxisListType.X)
                        asmf = asm[:].bitcast(f32)
                        nc.vector.tensor_copy(out=asmf[0:R, 49:50], in_=q[0:R, :])
                        nc.vector.tensor_copy(out=asmf[0:R, 50:54], in_=x_t[0:R, :])
                        nc.sync.dma_start(out=Bsh[b0:b0 + R, :], in_=asm[0:R, :])

                        # Aqx: [q | eta*x | pad]
                        aqx = sb.tile([128, 8], f32, tag="p0_aqx")
                        nc.vector.memset(aqx[:], 0.0)
                        nc.vector.tensor_copy(out=aqx[0:R, 0:1], in_=q[0:R, :])
                        nc.vector.tensor_copy(out=aqx[0:R, 1:5], in_=xe[0:R, :])
                        nc.sync.dma_start(out=Aqx[b0:b0 + R, :], in_=aqx[0:R, :])

            nc.gpsimd.collective_compute(
                "AllGather", ALU.bypass,
                ins=[Bsh[:, :]], outs=[Btab[:, :]],
                replica_groups=[list(range(ncores))],
            )

            # ================= P1: edge blocks =================
            for b in range(nb):
                agx = sb.tile([128, EB // 16], i16, tag="agx")
                nc.sync.dma_start(out=agx[:], in_=agidx_in[b, :, :])
                meta_t = sb.tile([128, 9], i32, tag="meta")
                nc.sync.dma_start(out=meta_t[:], in_=meta_in[b, :, :])
                loc_t = sb.tile([128, NP], f32, tag="loc")
                nc.sync.dma_start(out=loc_t[:], in_=localf_in[b, :, :])

                gA = sb.tile([128, EB], bf16, tag="gA")
                nc.gpsimd.dma_gather(
                    out_ap=gA[:].rearrange("p (o n) -> p o n", o=1),
                    in_ap=Atab[:, :], idxs_ap=agx[:],
                    num_idxs=EB, num_idxs_reg=EB, elem_size=128, transpose=True)

                gB = sb.tile([128, NP * 112], bf16, tag="gB")
                gB3 = gB[:].rearrange("p (k c) -> p k c", k=NP)
                gq = sb.tile([128, NP * 8], f32, tag="gq")
                gq3 = gq[:].rearrange("p (k c) -> p k c", k=NP)
                for p in range(NP):
                    nc.gpsimd.indirect_dma_start(
                        out=gB3[:, p, :], out_offset=None, in_=Btab[:, :],
                        in_offset=bass.IndirectOffsetOnAxis(
                            ap=meta_t[:, p:p + 1], axis=0))
                    nc.gpsimd.indirect_dma_start(
                        out=gq3[:, p, :], out_offset=None, in_=Aqx[:, :],
                        in_offset=bass.IndirectOffsetOnAxis(
                            ap=meta_t[:, 4 + p:5 + p], axis=0))

                # Minkowski invariants, edge-major, all 4 pieces batched
                gBf = gB[:].bitcast(f32).rearrange("p (k c) -> p k c", k=NP)
                tmp = sb.tile([128, NP * 4], f32, tag="tmp")
                tmp3 = tmp[:].rearrange("p (k c) -> p k c", k=NP)
                nc.vector.tensor_tensor(out=tmp3, in0=gq3[:, :, 1:5],
                                        in1=gBf[:, :, 50:54], op=ALU.mult)
                crosses = sb.tile([128, NP], f32, tag="crosses")
                nc.vector.reduce_sum(crosses[:], tmp3, axis=mybir.AxisListType.X)
                psin = sb.tile([128, 2 * NP], f32, tag="psin")
                # norms = q_i + q_j - 2*cross  -> psin[:, 0:4]
                nc.vector.tensor_tensor(out=psin[:, 4:8], in0=gq3[:, :, 0:1],
                                        in1=gBf[:, :, 49:50], op=ALU.add)
                nc.vector.tensor_scalar(out=psin[:, 0:4], in0=crosses[:],
                                        scalar1=-2.0, scalar2=None, op0=ALU.mult)
                nc.vector.tensor_tensor(out=psin[:, 0:4], in0=psin[:, 0:4],
                                        in1=psin[:, 4:8], op=ALU.add)
                # prods = cross -> psin[:, 4:8]
                nc.vector.tensor_copy(out=psin[:, 4:8], in_=crosses[:])
                # psi(v) = sign(v) * ln(|v| + 1)
                sgn = sb.tile([128, 2 * NP], f32, tag="sgn")
                nc.scalar.activation(out=sgn[:], in_=psin[:], func=AF.Sign)
                av = sb.tile([128, 2 * NP], f32, tag="av")
                nc.scalar.activation(out=av[:], in_=psin[:], func=AF.Abs)
                lnv = sb.tile([128, 2 * NP], f32, tag="lnv")
                nc.scalar.activation(out=lnv[:], in_=av[:], func=AF.Ln, bias=1.0)
                psio = sb.tile([128, 2 * NP], f32, tag="psio")
                nc.vector.tensor_tensor(out=psio[:], in0=sgn[:], in1=lnv[:],
                                        op=ALU.mult)
                # write psi pairs into gB pad cols 96:98 of each piece
                nc.vector.tensor_copy(
                    out=gB3[:, :, 96:98],
                    in_=psio[:].rearrange("p (two k) -> p k two", two=2))

                # ---- feature-major MLP
                pre = ps.tile([96, EB], f32, tag="ps1")
                nc.tensor.matmul(out=pre[:, :], lhsT=W["selA"][:],
                                 rhs=gA[:], start=True, stop=False)
                psb = ps.tile([2, EB], f32, tag="ps7t")
                pb0 = None
                for p in range(NP):
                    nc.tensor.matmul(
                        out=pre[:, p * 128:(p + 1) * 128],
                        lhsT=gB3[:, p, 0:96], rhs=identB[:],
                        start=False, stop=False)
                    inst = nc.tensor.matmul(
                        out=psb[:, p * 128:(p + 1) * 128],
                        lhsT=gB3[:, p, 96:98], rhs=identB[:],
                        start=(p == 0), stop=(p == NP - 1))
                    if p == 0:
                        pb0 = inst
                    else:
                        add_dep_helper(inst.ins, pb0.ins, sync=False,
                                       reason="psi bank start order")
                psiT = sb.tile([2, EB], bf16, tag="psiT")
                nc.vector.tensor_copy(out=psiT[:], in_=psb[:, :])
                nc.tensor.matmul(out=pre[:, :], lhsT=W["wmisc_ext"][:],
                                 rhs=psiT[:], start=False, stop=True)

                l1 = sb.tile([96, EB], f32, tag="l1")
                nc.scalar.activation(out=l1[:], in_=pre[0:96, :], func=AF.Relu)
                l2p = ps.tile([96, EB], f32, tag="ps2t")
                nc.tensor.matmul(out=l2p[:, :], lhsT=W["We2"][:], rhs=l1[:],
                                 start=True, stop=True)
                mgp = sb.tile([97, EB], f32, tag="mgp")
                nc.scalar.activation(out=mgp[0:96, :], in_=l2p[:, :], func=AF.Relu,
                                     bias=W["be2"][:])
                gup = ps.tile([97, EB], f32, tag="ps3t")
                nc.tensor.matmul(out=gup[:, :], lhsT=W["Wx1Wm"][:],
                                 rhs=mgp[0:96, :], start=True, stop=True)
                nc.scalar.activation(out=mgp[96:97, :], in_=gup[96:97, :],
                                     func=AF.Sigmoid, bias=W["bx1m"][96:97, :])
                u1 = sb.tile([96, EB], f32, tag="u1")
                nc.scalar.activation(out=u1[:], in_=gup[0:96, :], func=AF.Relu,
                                     bias=W["bx1m"][0:96, :])
                php = ps.tile([1, EB], f32, tag="ps4t")
                nc.tensor.matmul(out=php[:, :], lhsT=W["Wx2"][:], rhs=u1[:],
                                 start=True, stop=True)
                phi_sb = sb.tile([1, EB], f32, tag="phi_sb")
                nc.scalar.activation(out=phi_sb[:], in_=php[:, :], func=AF.Copy)

                # ---- back to edge-major
                mgpT = ps.tile([128, NP * 98], f32, tag="ps5t")
                tp0 = None
                for p in range(NP):
                    inst = nc.tensor.matmul(
                        out=mgpT[:, p * 98:p * 98 + 97],
                        lhsT=mgp[:, p * 128:(p + 1) * 128],
                        rhs=identF[0:97, 0:97], is_transpose=True,
                        start=(p == 0), stop=False)
                    if p == 0:
                        tp0 = inst
                    else:
                        add_dep_helper(inst.ins, tp0.ins, sync=False,
                                       reason="mgpT bank start order")
                    inst2 = nc.tensor.matmul(
                        out=mgpT[:, p * 98 + 97:p * 98 + 98],
                        lhsT=phi_sb[:, p * 128:(p + 1) * 128],
                        rhs=identF[0:1, 0:1], is_transpose=True,
                        start=False, stop=(p == NP - 1))
                    add_dep_helper(inst2.ins, tp0.ins, sync=False,
                                   reason="mgpT bank start order (phi)")
                mgpT3 = mgpT[:].rearrange("p (k c) -> p k c", k=NP)
                gp = sb.tile([128, NP * 2], f32, tag="gp")
                gp3 = gp[:].rearrange("p (k c) -> p k c", k=NP)
                nc.vector.tensor_copy(out=gp3, in_=mgpT3[:, :, 96:98])

                vals = sb.tile([128, NP * 101], f32, tag="vals")
                vals3 = vals[:].rearrange("p (k c) -> p k c", k=NP)
                nc.vector.tensor_tensor(
                    out=vals3[:, :, 0:96], in0=mgpT3[:, :, 0:96],
                    in1=gp3[:, :, 0:1].to_broadcast([128, NP, 96]), op=ALU.mult)
                nc.vector.tensor_tensor(
                    out=vals3[:, :, 96:100], in0=gBf[:, :, 50:54],
                    in1=gp3[:, :, 1:2].to_broadcast([128, NP, 4]), op=ALU.mult)
                nc.gpsimd.memset(vals3[:, :, 100:101], 1.0)

                st4 = sb.tile([128, NP * 128], f32, tag="st4")
                for p in range(NP):
                    nc.vector.tensor_tensor(
                        out=st4[:, p * 128:(p + 1) * 128],
                        in0=loc_t[:, p:p + 1].to_broadcast([128, 128]),
                        in1=W["iota_rows"][:], op=ALU.is_equal)

                acc = ps2.tile([128, 101], f32, tag="ps6t")
                for p in range(NP):
                    nc.tensor.matmul(
                        out=acc[:, :], lhsT=st4[:, p * 128:(p + 1) * 128],
                        rhs=vals[:, p * 101:(p + 1) * 101],
                        start=(p == 0), stop=(p == NP - 1))
                acc_sb = sb.tile([128, 101], f32, tag="acc_sb")
                nc.scalar.activation(out=acc_sb[:], in_=acc[:, :], func=AF.Copy)
                nc.gpsimd.indirect_dma_start(
                    out=wmsc[:, :],
                    out_offset=bass.IndirectOffsetOnAxis(ap=meta_t[:, 8:9], axis=0),
                    in_=acc_sb[:], in_offset=None,
                    bounds_check=nsh - 1, oob_is_err=False)

            # ================= P2: node update =================
            for t in range(ntiles):
                b0 = t * 128
                R = min(128, nsh - b0)
                wa = sb1.tile([128, 101], f32, tag="p2_wa")
                nc.sync.dma_start(out=wa[0:R, :], in_=wmsc[b0:b0 + R, :])
                hTt = sb1.tile([96, 128], f32, tag="p2_hT")
                nc.sync.dma_start(out=hTt[:, 0:R], in_=hT_dram[:, b0:b0 + R])
                h_t = sb1.tile([128, 96], f32, tag="p2_h")
                nc.sync.dma_start(out=h_t[0:R, :], in_=h_sh[b0:b0 + R, :])
                x_t = sb1.tile([128, 4], f32, tag="p2_x")
                nc.sync.dma_start(out=x_t[0:R, :], in_=x_sh[b0:b0 + R, :])

                cntc = sb1.tile([128, 1], f32, tag="p2_cnt")
                nc.vector.tensor_scalar_max(cntc[0:R, :], wa[0:R, 100:101], 1.0)
                rec = sb1.tile([128, 1], f32, tag="p2_rec")
                nc.vector.reciprocal(rec[0:R, :], cntc[0:R, :])
                mean = sb1.tile([128, 4], f32, tag="p2_mean")
                nc.vector.tensor_tensor(out=mean[0:R, :], in0=wa[0:R, 96:100],
                                        in1=rec[0:R, :].to_broadcast([R, 4]),
                                        op=ALU.mult)
                nc.vector.tensor_scalar_mul(mean[0:R, :], mean[0:R, :], C_CONST)
                xo = sb1.tile([128, 4], f32, tag="p2_xo")
                nc.vector.tensor_tensor(out=xo[0:R, :], in0=mean[0:R, :],
                                        in1=x_t[0:R, :], op=ALU.add)
                nc.sync.dma_start(out=x_out[b0:b0 + R, :], in_=xo[0:R, :])

                psW = ps.tile([96, 128], f32, tag="ps1")
                nc.tensor.matmul(out=psW[:, 0:R], lhsT=wa[0:R, 0:96],
                                 rhs=identF[0:R, 0:R], is_transpose=True,
                                 start=True, stop=True)
                wmT = sb1.tile([96, 128], f32, tag="p2_wmT")
                nc.scalar.activation(out=wmT[:, 0:R], in_=psW[:, 0:R], func=AF.Copy)

                psG = ps.tile([96, 128], f32, tag="ps2t")
                nc.tensor.matmul(out=psG[:, 0:R], lhsT=W["Wh1h"][:],
                                 rhs=hTt[:, 0:R], start=True, stop=False)
                nc.tensor.matmul(out=psG[:, 0:R], lhsT=W["Wh1wm"][:],
                                 rhs=wmT[:, 0:R], start=False, stop=True)
                g1 = sb1.tile([96, 128], f32, tag="p2_g1")
                nc.scalar.activation(out=g1[:, 0:R], in_=psG[:, 0:R],
                                     func=AF.Relu, bias=W["bh1"][:])
                psD = ps.tile([96, 128], f32, tag="ps3t")
                nc.tensor.matmul(out=psD[:, 0:R], lhsT=W["Wh2"][:],
                                 rhs=g1[:, 0:R], start=True, stop=True)
                dhT = sb1.tile([96, 128], f32, tag="p2_dhT")
                nc.scalar.activation(out=dhT[:, 0:R], in_=psD[:, 0:R],
                                     func=AF.Copy, bias=0.0)
                nc.vector.tensor_tensor(out=dhT[:, 0:R], in0=dhT[:, 0:R],
                                        in1=W["bh2"][:].to_broadcast([96, R]),
                                        op=ALU.add)
                psO = ps.tile([128, 96], f32, tag="ps4t")
                nc.tensor.matmul(out=psO[0:R, :], lhsT=dhT[:, 0:R],
                                 rhs=identF[0:96, 0:96], is_transpose=True,
                                 start=True, stop=True)
                ho = sb1.tile([128, 96], f32, tag="p2_ho")
                nc.vector.tensor_tensor(out=ho[0:R, :], in0=psO[0:R, :],
                                        in1=h_t[0:R, :], op=ALU.add)
                nc.sync.dma_start(out=h_out[b0:b0 + R, :], in_=ho[0:R, :])

    nc.compile()
    return nc


# ---------------------------------------------------------------- entry point

def kernel(x, h, edge_i, edge_j, We1, be1, We2, be2, Wm, bm,
           Wh1, bh1, Wh2, bh2, Wx1, bx1, Wx2):
    nc, in_maps, nsh = prepare(x, h, edge_i, edge_j, We1, be1, We2, be2,
                               Wm, bm, Wh1, bh1, Wh2, bh2, Wx1, bx1, Wx2)
    res = bass_utils.run_bass_kernel_spmd(nc, in_maps,
                                          core_ids=list(range(NCORES)))
    h_out = np.concatenate([res.results[c]["h_out"] for c in range(NCORES)], 0)
    x_out = np.concatenate([res.results[c]["x_out"] for c in range(NCORES)], 0)
    return (h_out, x_out)


# ------------------------------------------------------- timing helper (dev)

def timed_run(nc, in_maps, iters=10):
    """Compile once, execute `iters` times on the 8 cores with device-resident
    inputs; returns (results_list, best_wall_ns_per_iter)."""
    import time
    import jax
    from jax.sharding import Mesh, PartitionSpec, NamedSharding
    from jax.experimental.shard_map import shard_map
    import concourse.mybir as mybir_
    from concourse import bass2jax

    bass2jax.install_neuronx_cc_hook()
    n_cores = len(in_maps)
    partition_name = (nc.partition_id_tensor.name
                      if nc.partition_id_tensor else None)
    in_names, out_names, out_avals, zero_outs = [], [], [], []
    for alloc in nc.m.functions[0].allocations:
        if not isinstance(alloc, mybir_.MemoryLocationSet):
            continue
        name = alloc.memorylocations[0].name
        if alloc.kind == "ExternalInput":
            if name != partition_name:
                in_names.append(name)
        elif alloc.kind == "ExternalOutput":
            shape = tuple(alloc.tensor_shape)
            dtype = mybir_.dt.np(alloc.dtype)
            out_names.append(name)
            out_avals.append(jax.core.ShapedArray(shape, dtype))
            zero_outs.append(np.zeros(shape, dtype))
    n_params = len(in_names)
    all_in_names = list(in_names) + list(out_names)
    if partition_name is not None:
        all_in_names.append(partition_name)

    def _body(*args):
        operands = list(args)
        if partition_name is not None:
            operands.append(bass2jax.partition_id_tensor())
        outs = bass2jax._bass_exec_p.bind(
            *operands, out_avals=tuple(out_avals), in_names=tuple(all_in_names),
            out_names=tuple(out_names), lowering_input_output_aliases=(),
            sim_require_finite=True, sim_require_nnan=True, nc=nc)
        return tuple(outs)

    devices = jax.devices()[:n_cores]
    mesh = Mesh(np.asarray(devices), ("core",))
    nspec = (PartitionSpec("core"),)
    fn = jax.jit(shard_map(_body, mesh=mesh,
                           in_specs=nspec * (n_params + len(out_names)),
                           out_specs=nspec * len(out_names), check_rep=False),
                 keep_unused=True)
    concat_in = [np.concatenate([np.asarray(in_maps[c][k]).reshape(
                     in_maps[c][k].shape) for c in range(n_cores)], axis=0)
                 for k in in_names]
    concat_zero = [np.zeros((n_cores * z.shape[0], *z.shape[1:]), z.dtype)
                   for z in zero_outs]
    sh = NamedSharding(mesh, PartitionSpec("core"))
    dev_in = [jax.device_put(a, sh) for a in concat_in + concat_zero]
    outs = fn(*dev_in)
    jax.block_until_ready(outs)
    best = float("inf")
    for _ in range(iters):
        t0 = time.perf_counter()
        outs = fn(*dev_in)
        jax.block_until_ready(outs)
        best = min(best, time.perf_counter() - t0)
    res = []
    for c in range(n_cores):
        m = {}
        for i, name in enumerate(out_names):
            full = np.asarray(outs[i])
            rows = full.shape[0] // n_cores
            m[name] = full[c * rows:(c + 1) * rows]
        res.append(m)
    return res, best * 1e9


def prepare(x, h, edge_i, edge_j, We1, be1, We2, be2, Wm, bm,
            Wh1, bh1, Wh2, bh2, Wx1, bx1, Wx2):
    """Host prep + program build; returns (nc, in_maps, nsh)."""
    x = np.asarray(x, np.float32)
    h = np.asarray(h, np.float32)
    edge_i = np.asarray(edge_i)
    edge_j = np.asarray(edge_j)
    Wd = _pack_weights(dict(We1=np.asarray(We1), be1=np.asarray(be1),
                            We2=np.asarray(We2), be2=np.asarray(be2),
                            Wm=np.asarray(Wm), bm=np.asarray(bm),
                            Wh1=np.asarray(Wh1), bh1=np.asarray(bh1),
                            Wh2=np.asarray(Wh2), bh2=np.asarray(bh2),
                            Wx1=np.asarray(Wx1), bx1=np.asarray(bx1),
                            Wx2=np.asarray(Wx2)))
    nb, nsh, cores = build_host_data(x, h, edge_i, edge_j)
    nc = build_nc(nb, nsh)
    in_maps = []
    for c in range(NCORES):
        m = dict(h_sh=h[c * nsh:(c + 1) * nsh], x_sh=x[c * nsh:(c + 1) * nsh],
                 agidx=cores[c]["agidx"], meta=cores[c]["meta"],
                 localf=cores[c]["localf"])
        for name, shp, dt in WEIGHT_SPECS:
            m[name] = Wd[name]
        in_maps.append(m)
    return nc, in_maps, nsh
